# revision 59
# baseline (speedup 1.0000x reference)
"""Capsule-routing kernel for Trainium2, 8-core batch-parallel.

Reference computation (per example, In=4096, D=256, N=16, K=16, routings=3):
    u_hat = (x @ W).reshape(In, N, K)           # [In, 256] with m = n*16+k
    b = 0
    for j in range(3):
        c = softmax(b, axis=n)                   # [In, N]
        outputs = squash(sum_i c[i,n] u_hat[i,n,:])   # [N, K]
        if j < 2: b[i,n] = sum_k outputs[n,k] u_hat[i,n,k]

Device strategy per core (4 examples):
  - host supplies xT [2,128,4096] per example (d on partitions)
  - PE Form3: u_hat[i,m] f32 (stationary = xT 128x128 slices, rhs = W)
  - PE Form2: u_hatT[m,i] -> stored bf16 (only feeds b-update)
  - outputs-acc: 32 accumulating matmuls lhsT=c-tile [128,16], rhs=u_hat tile
  - b-update: 64 matmuls lhsT=u_hatT slice [128,128] bf16, rhs=S [128,16] bf16
  - softmax over n: exp (ScalarE) + segmented reduce + reciprocal (DVE)
"""

import sys
from contextlib import ExitStack

sys.path.insert(0, "/opt/trn_rl_repo")

import numpy as np

import concourse.bass as bass
import concourse.mybir as mybir
import concourse.tile as tile
from concourse import bacc
from concourse.bass_utils import run_bass_kernel_spmd

F32 = mybir.dt.float32
F32R = mybir.dt.float32r
BF16 = mybir.dt.bfloat16
U32 = mybir.dt.uint32

N_CORES = 8
B = 32
IN = 4096
D = 256
N = 16
K = 16
M = N * K  # 256
EPS = 1e-7


def build_kernel(n_ex=4, n_tiles=32, routings=3, ut_dtype="bf16", ut_bufs=2):
    """Build the per-core Bass module. In = n_tiles*128."""
    In = n_tiles * 128
    nc = bacc.Bacc("TRN2", target_bir_lowering=False, debug=False,
                   num_devices=N_CORES)

    # DRAM I/O
    xT_d = nc.dram_tensor("xT", [n_ex, 2, 128, In], F32R, kind="ExternalInput")
    Wt_d = nc.dram_tensor("Wt", [2, 128, M], F32R, kind="ExternalInput")
    ones16_d = nc.dram_tensor("ones16", [128, N], F32R, kind="ExternalInput")
    bmask_d = nc.dram_tensor("bmask", [N, M], F32, kind="ExternalInput")
    id16_d = nc.dram_tensor("id16", [N, N], F32, kind="ExternalInput")
    sel4_d = nc.dram_tensor("sel4", [128, N], F32R, kind="ExternalInput")
    out_d = nc.dram_tensor("out", [n_ex, N, K], F32, kind="ExternalOutput")

    with tile.TileContext(nc) as tc, ExitStack() as ctx:
        # ---- pools ----
        const_pool = ctx.enter_context(tc.tile_pool(name="consts", bufs=1))
        xT_pool = ctx.enter_context(tc.tile_pool(name="xT", bufs=2))
        u_pool = ctx.enter_context(tc.tile_pool(name="u", bufs=2))
        uT_pool = ctx.enter_context(tc.tile_pool(name="uT", bufs=ut_bufs))
        small_pool = ctx.enter_context(tc.tile_pool(name="small", bufs=4))
        out_pool = ctx.enter_context(tc.tile_pool(name="outstage", bufs=1))

        ps_u = ctx.enter_context(tc.tile_pool(name="ps_u", bufs=2, space="PSUM"))
        ps_uT = ctx.enter_context(tc.tile_pool(name="ps_uT", bufs=1, space="PSUM"))
        ps_acc = ctx.enter_context(tc.tile_pool(name="ps_acc", bufs=2, space="PSUM"))
        ps_b = ctx.enter_context(tc.tile_pool(name="ps_b", bufs=2, space="PSUM"))
        ps_s = ctx.enter_context(tc.tile_pool(name="ps_s", bufs=1, space="PSUM"))

        # ---- constants ----
        Wt = const_pool.tile([128, 2, M], F32R, tag="Wt")
        nc.sync.dma_start(Wt[:], Wt_d.ap().rearrange("c p m -> p c m"))
        ones16 = const_pool.tile([128, N], F32R, tag="ones16")
        nc.sync.dma_start(ones16[:], ones16_d[:])
        bmask = const_pool.tile([N, M], F32, tag="bmask")
        nc.sync.dma_start(bmask[:], bmask_d[:])
        id16 = const_pool.tile([N, N], F32, tag="id16")
        nc.sync.dma_start(id16[:], id16_d[:])
        sel4 = const_pool.tile([128, N], F32R, tag="sel4")
        nc.sync.dma_start(sel4[:], sel4_d[:])

        out_stage = out_pool.tile([N, n_ex * K], F32, tag="outst")
        eps_t = const_pool.tile([N, 1], F32, tag="eps")
        nc.vector.memset(eps_t[:], EPS)

        for e in range(n_ex):
            # ======== load xT ========
            xT = xT_pool.tile([128, 2, In], F32R, tag="xT")
            nc.sync.dma_start(xT[:, 0, :], xT_d[e, 0])
            nc.sync.dma_start(xT[:, 1, :], xT_d[e, 1])

            # ======== Form 3: u_hat[i, m], tiles of [128, 256] ========
            u_sb = u_pool.tile([128, n_tiles, M], F32R, tag="u")
            for tp in range(n_tiles // 2):
                pu = ps_u.tile([128, 2, M], F32, tag="ps_u")
                for half in range(2):
                    t = 2 * tp + half
                    for dc in range(2):
                        nc.tensor.matmul(
                            pu[:, half, :],
                            xT[:, dc, 128 * t:128 * (t + 1)],
                            Wt[:, dc, :],
                            start=(dc == 0), stop=(dc == 1),
                            skip_group_check=True)
                if tp % 2 == 0:
                    nc.scalar.copy(u_sb[:, 2 * tp:2 * tp + 2, :], pu[:])
                else:
                    nc.vector.tensor_copy(u_sb[:, 2 * tp:2 * tp + 2, :], pu[:])

            # ======== Form 2: u_hatT[m, i] in bf16 ========
            uT_dt = {"bf16": BF16, "f32": F32, "f32r": F32R}[ut_dtype]
            uT_sb = uT_pool.tile([128, 2, In], uT_dt, tag="uT")
            n_ch = In // 512
            for mt in range(2):
                for ch in range(n_ch):
                    puT = ps_uT.tile([128, 512], F32, tag="ps_uT")
                    for dc in range(2):
                        nc.tensor.matmul(
                            puT[:],
                            Wt[:, dc, 128 * mt:128 * (mt + 1)],
                            xT[:, dc, 512 * ch:512 * (ch + 1)],
                            start=(dc == 0), stop=(dc == 1))
                    if ch % 2 == 0:
                        nc.scalar.copy(uT_sb[:, mt, 512 * ch:512 * (ch + 1)],
                                       puT[:])
                    else:
                        nc.vector.tensor_copy(
                            uT_sb[:, mt, 512 * ch:512 * (ch + 1)], puT[:])

            # ======== routing ========
            c_all = small_pool.tile([128, n_tiles, N], F32R, tag="c_all")
            for j in range(routings):
                # --- outputs accumulation -> acc_ps [16, 256] ---
                acc_ps = ps_acc.tile([N, M], F32, tag="acc")
                for t in range(n_tiles):
                    lhsT = ones16[:] if j == 0 else c_all[:, t, :]
                    nc.tensor.matmul(acc_ps[:], lhsT, u_sb[:, t, :],
                                     start=(t == 0), stop=(t == n_tiles - 1))

                # --- squash ---
                o_full = small_pool.tile([N, M], F32, tag="o_full")
                nc.scalar.copy(o_full[:], acc_ps[:])
                om = small_pool.tile([N, M], F32, tag="om")
                nrm2 = small_pool.tile([N, 1], F32, tag="nrm2")
                sq = small_pool.tile([N, M], F32, tag="sq")
                nc.vector.tensor_mul(om[:], o_full[:], bmask[:])
                nc.scalar.activation(sq[:], om[:],
                                     mybir.ActivationFunctionType.Square,
                                     accum_out=nrm2[:])
                # rinv = 1/sqrt(nrm2 + eps) via bit-trick + 2 Newton steps
                # (keeps ScalarE on the exp_and_friends ACT table: no
                # table-reload thrash from Sqrt)
                A = mybir.AluOpType
                xe = small_pool.tile([N, 1], F32, tag="xe")
                nc.vector.tensor_scalar_add(xe[:], nrm2[:], EPS)
                sbits = small_pool.tile([N, 1], U32, tag="sbits")
                nc.vector.tensor_scalar(sbits[:], xe[:].bitcast(U32), 1, None,
                                        op0=A.logical_shift_right)
                ybits = small_pool.tile([N, 1], U32, tag="ybits")
                nc.vector.tensor_scalar(ybits[:], sbits[:], -1.0,
                                        float(0x5F3759DF),
                                        op0=A.mult, op1=A.add)
                y = ybits[:].bitcast(F32)
                t1 = small_pool.tile([N, 1], F32, tag="t1")
                t2 = small_pool.tile([N, 1], F32, tag="t2")
                rinv = small_pool.tile([N, 1], F32, tag="rinv")
                n_newton = 2 if j == routings - 1 else 1
                for it in range(n_newton):
                    nc.vector.tensor_mul(t1[:], xe[:], y)
                    nc.vector.tensor_mul(t2[:], t1[:], y)
                    nc.vector.tensor_scalar(t2[:], t2[:], -0.5, 1.5,
                                            op0=A.mult, op1=A.add)
                    dst = rinv if it == n_newton - 1 else small_pool.tile(
                        [N, 1], F32, tag="ynext")
                    nc.vector.tensor_mul(dst[:], t2[:], y)
                    y = dst[:]
                o_n = small_pool.tile([N, M], F32, tag="o_n")
                nc.vector.tensor_scalar_mul(o_n[:], om[:], rinv[:])

                if j == routings - 1:
                    # final extraction: out[n,k] = sum_g o_n[n, g*16+k]
                    nc.vector.tensor_reduce(
                        out_stage[:, K * e:K * (e + 1)],
                        o_n[:].rearrange("p (g k) -> p k g", k=K),
                        axis=mybir.AxisListType.X, op=mybir.AluOpType.add)
                    continue

                # --- S build: S[mc] = transpose(o_n[:, mc*128:...]) bf16 ---
                S = small_pool.tile([128, 2, N], uT_dt, tag="S")
                sps = ps_s.tile([128, 2, N], F32, tag="s_ps")
                for mc in range(2):
                    nc.tensor.transpose(sps[:, mc, :],
                                        o_n[:, 128 * mc:128 * (mc + 1)],
                                        id16[:])
                nc.scalar.copy(S[:], sps[:])

                # --- b update: b[i, (t,n)] = sum_m u_hat[i,m] S[m,n] ---
                b_ps = ps_b.tile([128, n_tiles * N], F32, tag="b_ps")
                for t in range(n_tiles):
                    for mc in range(2):
                        nc.tensor.matmul(
                            b_ps[:, N * t:N * (t + 1)],
                            uT_sb[:, mc, 128 * t:128 * (t + 1)],
                            S[:, mc, :],
                            start=(mc == 0), stop=(mc == 1),
                            skip_group_check=True)
                b_all = small_pool.tile([128, n_tiles, N], F32, tag="b_all")
                nc.scalar.copy(
                    b_all[:], b_ps[:].rearrange("p (t n) -> p t n", n=N))

                # --- softmax over n ---
                e_all = small_pool.tile([128, n_tiles, N], F32, tag="e_all")
                nc.scalar.activation(e_all[:], b_all[:],
                                     mybir.ActivationFunctionType.Exp)
                s_sum = small_pool.tile([128, n_tiles], F32, tag="s_sum")
                nc.vector.tensor_reduce(s_sum[:], e_all[:],
                                        axis=mybir.AxisListType.X,
                                        op=mybir.AluOpType.add)
                s_r = small_pool.tile([128, n_tiles], F32, tag="s_r")
                nc.vector.reciprocal(s_r[:], s_sum[:])
                nc.vector.tensor_mul(
                    c_all[:], e_all[:],
                    s_r[:].to_broadcast([128, n_tiles, N]))

        # ======== store outputs ========
        nc.sync.dma_start(out_d.ap().rearrange("e n k -> n e k"),
                          out_stage[:].rearrange("p (e k) -> p e k", k=K))

    nc.compile()
    return nc


_NC_CACHE = {}


def _get_nc(n_ex=4, n_tiles=32, routings=3, ut_dtype="f32r", ut_bufs=1):
    key = (n_ex, n_tiles, routings, ut_dtype, ut_bufs)
    if key not in _NC_CACHE:
        _NC_CACHE[key] = build_kernel(*key)
    return _NC_CACHE[key]


def make_const_inputs():
    ones16 = np.full((128, N), 1.0 / N, dtype=np.float32)
    bmask = np.zeros((N, M), dtype=np.float32)
    for n in range(N):
        bmask[n, n * K:(n + 1) * K] = 1.0
    id16 = np.eye(N, dtype=np.float32)
    sel4 = np.zeros((128, N), dtype=np.float32)
    for p in range(128):
        if p % 32 < N:
            sel4[p, p % 32] = 1.0
    return ones16, bmask, id16, sel4


def kernel(x, W, num_capsule=None, dim_capsule=None, routings=None, **_):
    x = np.asarray(x, dtype=np.float32)
    W = np.asarray(W, dtype=np.float32)
    assert x.shape == (B, IN, D), x.shape

    nc = _get_nc()
    ones16, bmask, id16, sel4 = make_const_inputs()
    Wt = np.ascontiguousarray(W[0].reshape(2, 128, M))

    n_per = B // N_CORES
    in_maps = []
    for c in range(N_CORES):
        xs = x[c * n_per:(c + 1) * n_per]              # [4, 4096, 256]
        xT = np.ascontiguousarray(
            xs.transpose(0, 2, 1)).reshape(n_per, 2, 128, IN)
        in_maps.append({"xT": xT, "Wt": Wt, "ones16": ones16,
                        "bmask": bmask, "id16": id16, "sel4": sel4})

    res = run_bass_kernel_spmd(nc, in_maps, core_ids=list(range(N_CORES)))
    out = np.concatenate([r["out"] for r in res.results], axis=0)
    return out.astype(np.float32)

